# revision 21
# baseline (speedup 1.0000x reference)
"""CIN (xDeepFM compressed interaction network) kernel for Trainium2.

Reference computation (per batch b, embedding dim d):
  h1[b,h,d] = sum_{i,j} x[b,i,d] * x[b,j,d]  * W0[i*39+j, h]   i,j < 39
  h2[b,h,d] = sum_{i,j} x[b,i,d] * h1[b,j,d] * W1[i*128+j, h]  i < 39, j < 128
  h3[b,h,d] = sum_{i,j} x[b,i,d] * h2[b,j,d] * W2[i*128+j, h]
  out[b, :] = concat(sum_d h1, sum_d h2, sum_d h3)   -> [B, 384]

Strategy: data-parallel over batch on 8 cores (512 batches each). On-chip
layout is feature-on-partitions / (b,d)-on-free-dim, fp16 compute with fp32
PSUM accumulation, fully fused across the three layers (h1/h2 never touch
HBM). Outer-product "Z" tiles are built with DVE elementwise multiplies
against x-rows replicated across partitions via DMA broadcast from a DRAM
scratch copy of x^T. Layer 3 is never materialized: since only sum_d h3 is
needed, we compute per-batch Gram matrices G2[b,i,j] = sum_d x_i h2_j with
small PE transposes + matmuls and contract them with W2 once at the end.
"""

import sys

sys.path.insert(0, "/opt/trn_rl_repo")

import numpy as np

M = 39          # fields
D = 64          # embedding dim
H = 128         # hidden per CIN layer
B_TOTAL = 4096
N_CORES = 8
B_CORE = B_TOTAL // N_CORES      # 512 batches per core
TILE_B = 8                       # batches per tile
TILE_N = TILE_B * D              # 512 columns per tile
L1_CHUNK = 117                   # 39 j-values * 3 i-values (r-major: p = 3j+g)
L1_NCHUNK = (M * M) // L1_CHUNK  # 13
TT_G = 3                         # free-dim grouping of DVE multiplies

_NC_CACHE = {}


def _build(b_core):
    import concourse.bacc as bacc
    import concourse.tile as tile
    from concourse import mybir
    from concourse.masks import make_identity

    f32 = mybir.dt.float32
    f16 = mybir.dt.float16

    n_tiles = b_core // TILE_B
    bd = b_core * D

    nc = bacc.Bacc("TRN2", target_bir_lowering=False, debug=False)
    # host-prepared fp16 copies of x^T (plain and interleave-major for the
    # L1 chunk pattern); building these on-device would serialize ~100us of
    # DMA at kernel start
    xt16_d = nc.dram_tensor(
        "xt16", [n_tiles, M, TILE_N], f16, kind="ExternalInput"
    )
    xtg_d = nc.dram_tensor(
        "xtg", [n_tiles, 3, L1_NCHUNK, TILE_N], f16, kind="ExternalInput"
    )
    w0_d = nc.dram_tensor("W0", [M * M, H], f32, kind="ExternalInput")
    w1_d = nc.dram_tensor("W1", [M * H, H], f32, kind="ExternalInput")
    w2_d = nc.dram_tensor("W2", [M * H, H], f32, kind="ExternalInput")
    out_d = nc.dram_tensor("out", [3, H, b_core], f32, kind="ExternalOutput")

    with tile.TileContext(nc) as tc:
        with tc.tile_pool(name="resident", bufs=1) as resident:
            # ---- persistent weight tiles (fp16, pre-arranged as lhsT) ----
            # L1 chunk k, partition p: g = p%3, j = p//3, i = 3k+g,
            # row c = i*39+j
            w0_sb = resident.tile([L1_CHUNK, L1_NCHUNK, H], f16)
            w0_kg = w0_d.ap().rearrange("(k g j) h -> j k g h", k=L1_NCHUNK, g=3)
            for k in range(L1_NCHUNK):
                nc.gpsimd.dma_start(w0_sb[:, k, :], w0_kg[:, k, :, :])
            w1_sb = resident.tile([H, M, H], f16)
            nc.gpsimd.dma_start(
                w1_sb[:], w1_d.ap().rearrange("(i j) h -> j i h", j=H)
            )
            w2_sb = resident.tile([H, M, H], f16)
            nc.gpsimd.dma_start(
                w2_sb[:], w2_d.ap().rearrange("(i j) h -> j i h", j=H)
            )
            identity = resident.tile([H, H], f16)
            make_identity(nc, identity[:])


            # ---- per-core accumulated pooled outputs ----
            out_sb = resident.tile([H, 2, b_core], f32)
            g2t_sb = resident.tile([H, M, b_core], f16)

            xt16_ap = xt16_d.ap()  # [n_tiles, M, TILE_N], tile-major
            with (
                tc.tile_pool(name="pat", bufs=2) as pat,
                tc.tile_pool(name="patip", bufs=2) as patip,
                tc.tile_pool(name="zpool", bufs=4) as zpool,
                tc.tile_pool(name="hsb", bufs=2) as hsb,
                tc.tile_pool(name="gram", bufs=2) as gram,
                tc.tile_pool(name="psum", bufs=2, space="PSUM") as psum,
                tc.tile_pool(name="psum_t", bufs=1, space="PSUM") as psum_t,
            ):
                # ---- layer 3 via per-batch Gram matrices, deferred by one
                # tile and spliced piecewise into the next tile's layer-2
                # matmul stream so the PE never idles long enough to lose
                # its HAM clock boost ----
                def gram_pieces(t, xt_t, h2_16):
                    # G2T[j, i | b] = sum_d h2[j, d] * x[i, d]
                    xdt = gram.tile([D, TILE_B, M], f16, tag="xdt")

                    def xdt_piece(c):
                        cs = slice(c * D, (c + 1) * D)
                        xdt_ps = psum_t.tile([D, M], f16, tag="xdtps")
                        nc.tensor.transpose(
                            xdt_ps[:], xt_t[:, cs], identity[:M, :M]
                        )
                        nc.scalar.copy(xdt[:, c, :], xdt_ps[:])

                    def b_piece(b):
                        bs = slice(b * D, (b + 1) * D)
                        h2dt_ps = psum_t.tile([D, H], f16, tag="h2dtps")
                        nc.tensor.transpose(
                            h2dt_ps[:], h2_16[:, bs], identity[:]
                        )
                        h2dt = gram.tile([D, H], f16, tag="h2dt")
                        nc.scalar.copy(h2dt[:], h2dt_ps[:])
                        g2t_ps = psum_t.tile([H, M], f32, tag="g2tps")
                        nc.tensor.matmul(
                            g2t_ps[:], h2dt[:], xdt[:, b, :],
                            start=True, stop=True,
                        )
                        nc.scalar.copy(
                            g2t_sb[:, :, t * TILE_B + b], g2t_ps[:]
                        )

                    for c in range(0, TILE_B, 2):
                        yield lambda c=c: (xdt_piece(c), xdt_piece(c + 1))
                    for b in range(TILE_B):
                        yield lambda b=b: b_piece(b)

                def gram_phase(t, xt_t, h2_16):
                    for piece in gram_pieces(t, xt_t, h2_16):
                        piece()

                prev_gram = None
                for t in range(n_tiles):
                    ts = slice(t * TILE_N, (t + 1) * TILE_N)
                    # x rows replicated across partitions:
                    # bcast[p, i, :] = x^T[i, ts] for all p
                    bcast = pat.tile([H, M, TILE_N], f16)
                    nc.sync.dma_start(
                        bcast[:],
                        xt16_ap[t]
                        .rearrange("i c -> (i c)")[None]
                        .to_broadcast([H, M * TILE_N]),
                    )
                    # x_ip[p, k, :] = x^T[3k + p%3, ts]
                    x_ip = patip.tile([L1_CHUNK, L1_NCHUNK, TILE_N], f16)
                    nc.sync.dma_start(
                        x_ip[:],
                        xtg_d.ap()[t]
                        .rearrange("g k c -> g (k c)")[None]
                        .to_broadcast([M, 3, L1_NCHUNK * TILE_N]),
                    )
                    # x_jp[p, :] = x^T[p//3, ts]
                    x_jp = pat.tile([L1_CHUNK, TILE_N], f16)
                    nc.sync.dma_start(
                        x_jp[:],
                        xt16_ap[t][:, None, :].to_broadcast([M, 3, TILE_N]),
                    )
                    # plain x^T tile (fp16) for the layer-3 Gram transposes
                    xt_t = pat.tile([M, TILE_N], f16)
                    nc.sync.dma_start(xt_t[:], xt16_ap[t])

                    if prev_gram is not None:
                        gram_phase(*prev_gram)
                    prev_gram = None

                    # ---- layer 1 ----
                    h1_ps = psum.tile([H, TILE_N], f32, tag="h1ps")
                    for k0 in range(0, L1_NCHUNK, TT_G):
                        g = min(TT_G, L1_NCHUNK - k0)
                        z1 = zpool.tile([L1_CHUNK, TT_G, TILE_N], f16, tag="z1")
                        nc.vector.tensor_mul(
                            z1[:, :g, :],
                            x_ip[:, k0 : k0 + g, :],
                            x_jp[:, None, :].broadcast_to(
                                [L1_CHUNK, g, TILE_N]
                            ),
                        )
                        for u in range(g):
                            k = k0 + u
                            nc.tensor.matmul(
                                h1_ps[:],
                                w0_sb[:, k, :],
                                z1[:, u, :],
                                start=(k == 0),
                                stop=(k == L1_NCHUNK - 1),
                            )
                    h1_16 = hsb.tile([H, TILE_N], f16, tag="h1")
                    nc.scalar.copy(h1_16[:], h1_ps[:])
                    nc.vector.tensor_reduce(
                        out_sb[:, 0, t * TILE_B : (t + 1) * TILE_B],
                        h1_ps[:].rearrange("h (b d) -> h b d", d=D),
                        axis=mybir.AxisListType.X,
                        op=mybir.AluOpType.add,
                    )

                    # ---- layer 2 (with previous tile's Gram work spliced
                    # between matmul groups to keep the PE dense) ----
                    h2_ps = psum.tile([H, TILE_N], f32, tag="h2ps")
                    for gi, i0 in enumerate(range(0, M, TT_G)):
                        g = min(TT_G, M - i0)
                        z2 = zpool.tile([H, TT_G, TILE_N], f16, tag="z2")
                        nc.vector.tensor_mul(
                            z2[:, :g, :],
                            bcast[:, i0 : i0 + g, :],
                            h1_16[:, None, :].broadcast_to([H, g, TILE_N]),
                        )
                        for u in range(g):
                            i = i0 + u
                            nc.tensor.matmul(
                                h2_ps[:],
                                w1_sb[:, i, :],
                                z2[:, u, :],
                                start=(i == 0),
                                stop=(i == M - 1),
                            )
                    h2_16 = hsb.tile([H, TILE_N], f16, tag="h2")
                    nc.scalar.copy(h2_16[:], h2_ps[:])
                    nc.vector.tensor_reduce(
                        out_sb[:, 1, t * TILE_B : (t + 1) * TILE_B],
                        h2_ps[:].rearrange("h (b d) -> h b d", d=D),
                        axis=mybir.AxisListType.X,
                        op=mybir.AluOpType.add,
                    )

                    prev_gram = (t, xt_t, h2_16)

                gram_phase(*prev_gram)

                # ---- final contraction: out3 = W2^T @ G2T ----
                out3_ps = psum_t.tile([H, b_core], f32, tag="out3")
                for i in range(M):
                    nc.tensor.matmul(
                        out3_ps[:],
                        w2_sb[:, i, :],
                        g2t_sb[:, i, :],
                        start=(i == 0),
                        stop=(i == M - 1),
                    )
                out3_sb = resident.tile([H, b_core], f32)
                nc.vector.tensor_copy(out3_sb[:], out3_ps[:])

            nc.sync.dma_start(
                out_d.ap()[0:2].rearrange("l h b -> h l b"), out_sb[:]
            )
            nc.sync.dma_start(out_d.ap()[2], out3_sb[:])
    nc.compile()
    return nc


def _get_nc(b_core):
    if b_core not in _NC_CACHE:
        _NC_CACHE[b_core] = _build(b_core)
    return _NC_CACHE[b_core]


def kernel(x, W0, W1, W2, _trace=False):
    from concourse.bass_utils import run_bass_kernel_spmd

    x = np.ascontiguousarray(x, dtype=np.float32)
    w0 = np.ascontiguousarray(W0.reshape(M * M, H), dtype=np.float32)
    w1 = np.ascontiguousarray(W1.reshape(M * H, H), dtype=np.float32)
    w2 = np.ascontiguousarray(W2.reshape(M * H, H), dtype=np.float32)

    nc = _get_nc(B_CORE)
    n_tiles = B_CORE // TILE_B
    bd = B_CORE * D
    in_maps = []
    for c in range(N_CORES):
        xc = x[c * B_CORE : (c + 1) * B_CORE]
        xtr = xc.transpose(1, 0, 2).reshape(M, bd).astype(np.float16)
        xt16 = np.ascontiguousarray(
            xtr.reshape(M, n_tiles, TILE_N).transpose(1, 0, 2)
        )
        # xtg[t, g, k, c] = xt16[3k+g, t*TILE_N+c]
        xtg = np.ascontiguousarray(
            xtr.reshape(L1_NCHUNK, 3, n_tiles, TILE_N).transpose(2, 1, 0, 3)
        )
        in_maps.append(
            {"xt16": xt16, "xtg": xtg, "W0": w0, "W1": w1, "W2": w2}
        )
    res = run_bass_kernel_spmd(
        nc, in_maps, core_ids=list(range(N_CORES)), trace=_trace
    )
    # per-core out: [3, H, B_CORE] -> [B_CORE, 3*H]
    outs = []
    for c in range(N_CORES):
        o = res.results[c]["out"]  # [3, 128, 512]
        outs.append(o.reshape(3 * H, B_CORE).T.reshape(B_CORE, 3 * H))
    full = np.concatenate(outs, axis=0).astype(np.float32)
    if _trace:
        return full, res
    return full


# revision 23
# speedup vs baseline: 1.1527x; 1.1527x over previous
"""CIN (xDeepFM compressed interaction network) kernel for Trainium2.

Reference computation (per batch b, embedding dim d):
  h1[b,h,d] = sum_{i,j} x[b,i,d] * x[b,j,d]  * W0[i*39+j, h]   i,j < 39
  h2[b,h,d] = sum_{i,j} x[b,i,d] * h1[b,j,d] * W1[i*128+j, h]  i < 39, j < 128
  h3[b,h,d] = sum_{i,j} x[b,i,d] * h2[b,j,d] * W2[i*128+j, h]
  out[b, :] = concat(sum_d h1, sum_d h2, sum_d h3)   -> [B, 384]

Strategy: data-parallel over batch on 8 cores (512 batches each). On-chip
layout is feature-on-partitions / (b,d)-on-free-dim, fp16 compute with fp32
PSUM accumulation, fully fused across the three layers (h1/h2 never touch
HBM).

Layer 1 exploits symmetry of x (x) x: W0 is folded host-side to the upper
triangle (780 pairs, padded to 117x7), so layer 1 costs 7 matmul passes
instead of 13. Its elementwise operand patterns are host-packed.

Layer 2 builds outer-product "Z" tiles with DVE fp16 multiplies against
x-rows replicated across 128 partitions by a DMA broadcast whose source is
one contiguous run per replica.

Layer 3 is never materialized: only sum_d h3 is needed, so per-batch Gram
matrices G2[b,j,i] = sum_d h2_j x_i are formed with small PE transposes +
matmuls (deferred one tile to overlap the pipeline bubble) and contracted
with W2 once at the end.
"""

import sys

sys.path.insert(0, "/opt/trn_rl_repo")

import numpy as np

M = 39          # fields
D = 64          # embedding dim
H = 128         # hidden per CIN layer
B_TOTAL = 4096
N_CORES = 8
B_CORE = B_TOTAL // N_CORES      # 512 batches per core
TILE_B = 8                       # batches per tile
TILE_N = TILE_B * D              # 512 columns per tile
L1_CHUNK = 117                   # partition rows per layer-1 chunk
L1_K = 7                         # layer-1 chunks (117*7 = 819 >= 780 pairs)
TT_G = 3                         # free-dim grouping of DVE multiplies

_NC_CACHE = {}

# upper-triangle pair enumeration for layer 1, row-major into [117, 7]
_PAIRS = [(i, j) for i in range(M) for j in range(i, M)]  # 780


def _build(b_core):
    import concourse.bacc as bacc
    import concourse.tile as tile
    from concourse import mybir
    from concourse.masks import make_identity

    f32 = mybir.dt.float32
    f16 = mybir.dt.float16

    n_tiles = b_core // TILE_B

    nc = bacc.Bacc("TRN2", target_bir_lowering=False, debug=False)
    # host-prepared tensors (fp16, pre-arranged); see kernel() below
    xt16_d = nc.dram_tensor(
        "xt16", [n_tiles, M, TILE_N], f16, kind="ExternalInput"
    )
    xip_d = nc.dram_tensor(
        "xip", [n_tiles, L1_CHUNK, L1_K, TILE_N], f16, kind="ExternalInput"
    )
    xjp_d = nc.dram_tensor(
        "xjp", [n_tiles, L1_CHUNK, L1_K, TILE_N], f16, kind="ExternalInput"
    )
    w0_d = nc.dram_tensor(
        "W0s", [L1_CHUNK, L1_K, H], f16, kind="ExternalInput"
    )
    w1_d = nc.dram_tensor("W1t", [H, M, H], f16, kind="ExternalInput")
    w2_d = nc.dram_tensor("W2t", [H, M, H], f16, kind="ExternalInput")
    out_d = nc.dram_tensor("out", [3, H, b_core], f32, kind="ExternalOutput")

    with tile.TileContext(nc) as tc:
        with tc.tile_pool(name="resident", bufs=1) as resident:
            w0_sb = resident.tile([L1_CHUNK, L1_K, H], f16)
            nc.sync.dma_start(w0_sb[:], w0_d.ap())
            w1_sb = resident.tile([H, M, H], f16)
            nc.sync.dma_start(w1_sb[:], w1_d.ap())
            w2_sb = resident.tile([H, M, H], f16)
            nc.sync.dma_start(w2_sb[:], w2_d.ap())
            identity = resident.tile([H, H], f16)
            make_identity(nc, identity[:])

            # per-core accumulated outputs
            out_sb = resident.tile([H, 2, b_core], f32)
            g2t_sb = resident.tile([H, M, b_core], f16)

            xt16_ap = xt16_d.ap()  # [n_tiles, M, TILE_N], tile-major
            with (
                tc.tile_pool(name="pat", bufs=2) as pat,
                tc.tile_pool(name="patip", bufs=2) as patip,
                tc.tile_pool(name="zpool", bufs=4) as zpool,
                tc.tile_pool(name="hsb", bufs=2) as hsb,
                tc.tile_pool(name="gram", bufs=2) as gram,
                tc.tile_pool(name="psum", bufs=2, space="PSUM") as psum,
                tc.tile_pool(name="psum_t", bufs=1, space="PSUM") as psum_t,
            ):
                # layer 3 via per-batch Gram matrices, deferred by one tile
                # so its PE work fills the bubble while the next tile's Z
                # tiles are being built on DVE
                def gram_phase(t, xt_t, h2_16):
                    # G2T[j, i | b] = sum_d h2[j, d] * x[i, d]
                    xdt = gram.tile([D, TILE_B, M], f16, tag="xdt")
                    for c in range(TILE_B):
                        cs = slice(c * D, (c + 1) * D)
                        xdt_ps = psum_t.tile([D, M], f16, tag="xdtps")
                        nc.tensor.transpose(
                            xdt_ps[:], xt_t[:, cs], identity[:M, :M]
                        )
                        nc.scalar.copy(xdt[:, c, :], xdt_ps[:])
                    for b in range(TILE_B):
                        bs = slice(b * D, (b + 1) * D)
                        h2dt_ps = psum_t.tile([D, H], f16, tag="h2dtps")
                        nc.tensor.transpose(
                            h2dt_ps[:], h2_16[:, bs], identity[:]
                        )
                        h2dt = gram.tile([D, H], f16, tag="h2dt")
                        nc.scalar.copy(h2dt[:], h2dt_ps[:])
                        g2t_ps = psum_t.tile([H, M], f32, tag="g2tps")
                        nc.tensor.matmul(
                            g2t_ps[:], h2dt[:], xdt[:, b, :],
                            start=True, stop=True,
                        )
                        nc.scalar.copy(
                            g2t_sb[:, :, t * TILE_B + b], g2t_ps[:]
                        )

                prev_gram = None
                for t in range(n_tiles):
                    # x rows replicated across partitions:
                    # bcast[p, i, :] = x^T[i, tile t] for all p
                    bcast = pat.tile([H, M, TILE_N], f16)
                    nc.sync.dma_start(
                        bcast[:],
                        xt16_ap[t]
                        .rearrange("i c -> (i c)")[None]
                        .to_broadcast([H, M * TILE_N]),
                    )
                    # host-packed layer-1 operand patterns
                    x_ip = patip.tile([L1_CHUNK, L1_K, TILE_N], f16, tag="ip")
                    nc.sync.dma_start(x_ip[:], xip_d.ap()[t])
                    x_jp = patip.tile([L1_CHUNK, L1_K, TILE_N], f16, tag="jp")
                    nc.sync.dma_start(x_jp[:], xjp_d.ap()[t])
                    # plain x^T tile for the layer-3 Gram transposes
                    xt_t = pat.tile([M, TILE_N], f16)
                    nc.sync.dma_start(xt_t[:], xt16_ap[t])

                    # deferred layer-3 of the previous tile
                    if prev_gram is not None:
                        gram_phase(*prev_gram)
                    prev_gram = None

                    # ---- layer 1 (symmetrized) ----
                    h1_ps = psum.tile([H, TILE_N], f32, tag="h1ps")
                    for k0 in range(0, L1_K, TT_G):
                        g = min(TT_G, L1_K - k0)
                        z1 = zpool.tile([L1_CHUNK, TT_G, TILE_N], f16, tag="z1")
                        nc.vector.tensor_mul(
                            z1[:, :g, :],
                            x_ip[:, k0 : k0 + g, :],
                            x_jp[:, k0 : k0 + g, :],
                        )
                        for u in range(g):
                            k = k0 + u
                            nc.tensor.matmul(
                                h1_ps[:],
                                w0_sb[:, k, :],
                                z1[:, u, :],
                                start=(k == 0),
                                stop=(k == L1_K - 1),
                            )
                    h1_16 = hsb.tile([H, TILE_N], f16, tag="h1")
                    for b in range(TILE_B):
                        bs = slice(b * D, (b + 1) * D)
                        nc.scalar.activation(
                            h1_16[:, bs],
                            h1_ps[:, bs],
                            mybir.ActivationFunctionType.Copy,
                            accum_out=out_sb[
                                :, 0, t * TILE_B + b : t * TILE_B + b + 1
                            ],
                        )

                    # ---- layer 2 ----
                    h2_ps = psum.tile([H, TILE_N], f32, tag="h2ps")
                    for i0 in range(0, M, TT_G):
                        g = min(TT_G, M - i0)
                        z2 = zpool.tile([H, TT_G, TILE_N], f16, tag="z2")
                        nc.vector.tensor_mul(
                            z2[:, :g, :],
                            bcast[:, i0 : i0 + g, :],
                            h1_16[:, None, :].broadcast_to([H, g, TILE_N]),
                        )
                        for u in range(g):
                            i = i0 + u
                            nc.tensor.matmul(
                                h2_ps[:],
                                w1_sb[:, i, :],
                                z2[:, u, :],
                                start=(i == 0),
                                stop=(i == M - 1),
                            )
                    h2_16 = hsb.tile([H, TILE_N], f16, tag="h2")
                    for b in range(TILE_B):
                        bs = slice(b * D, (b + 1) * D)
                        nc.scalar.activation(
                            h2_16[:, bs],
                            h2_ps[:, bs],
                            mybir.ActivationFunctionType.Copy,
                            accum_out=out_sb[
                                :, 1, t * TILE_B + b : t * TILE_B + b + 1
                            ],
                        )

                    prev_gram = (t, xt_t, h2_16)

                gram_phase(*prev_gram)

                # ---- final contraction: out3 = W2^T @ G2T ----
                out3_ps = psum_t.tile([H, b_core], f32, tag="out3")
                for i in range(M):
                    nc.tensor.matmul(
                        out3_ps[:],
                        w2_sb[:, i, :],
                        g2t_sb[:, i, :],
                        start=(i == 0),
                        stop=(i == M - 1),
                    )
                out3_sb = resident.tile([H, b_core], f32)
                nc.vector.tensor_copy(out3_sb[:], out3_ps[:])

            nc.sync.dma_start(
                out_d.ap()[0:2].rearrange("l h b -> h l b"), out_sb[:]
            )
            nc.sync.dma_start(out_d.ap()[2], out3_sb[:])
    nc.compile()
    return nc


def _get_nc(b_core):
    if b_core not in _NC_CACHE:
        _NC_CACHE[b_core] = _build(b_core)
    return _NC_CACHE[b_core]


_IDX = None


def _pair_index():
    global _IDX
    if _IDX is None:
        ii = np.zeros(L1_CHUNK * L1_K, np.int64)
        jj = np.zeros(L1_CHUNK * L1_K, np.int64)
        for idx, (i, j) in enumerate(_PAIRS):
            ii[idx], jj[idx] = i, j
        _IDX = (ii, jj)
    return _IDX


def _pack_weights(W0, W1, W2):
    w0r = W0.reshape(M, M, H).astype(np.float32)
    w0s = np.zeros((L1_CHUNK * L1_K, H), np.float32)
    for idx, (i, j) in enumerate(_PAIRS):
        w0s[idx] = w0r[i, j] + (w0r[j, i] if i != j else 0.0)
    w0s = w0s.reshape(L1_CHUNK, L1_K, H).astype(np.float16)
    w1t = np.ascontiguousarray(
        W1.reshape(M, H, H).transpose(1, 0, 2)
    ).astype(np.float16)
    w2t = np.ascontiguousarray(
        W2.reshape(M, H, H).transpose(1, 0, 2)
    ).astype(np.float16)
    return w0s, w1t, w2t


def kernel(x, W0, W1, W2, _trace=False):
    from concourse.bass_utils import run_bass_kernel_spmd

    x = np.ascontiguousarray(x, dtype=np.float32)
    w0s, w1t, w2t = _pack_weights(W0, W1, W2)

    nc = _get_nc(B_CORE)
    n_tiles = B_CORE // TILE_B
    bd = B_CORE * D
    ii, jj = _pair_index()
    in_maps = []
    for c in range(N_CORES):
        xc = x[c * B_CORE : (c + 1) * B_CORE]
        xtr = xc.transpose(1, 0, 2).reshape(M, bd).astype(np.float16)
        xt16t = np.ascontiguousarray(
            xtr.reshape(M, n_tiles, TILE_N).transpose(1, 0, 2)
        )  # [n_tiles, M, TILE_N]
        xip = np.ascontiguousarray(
            xt16t[:, ii, :].reshape(n_tiles, L1_CHUNK, L1_K, TILE_N)
        )
        xjp = np.ascontiguousarray(
            xt16t[:, jj, :].reshape(n_tiles, L1_CHUNK, L1_K, TILE_N)
        )
        in_maps.append(
            {
                "xt16": xt16t,
                "xip": xip,
                "xjp": xjp,
                "W0s": w0s,
                "W1t": w1t,
                "W2t": w2t,
            }
        )
    res = run_bass_kernel_spmd(
        nc, in_maps, core_ids=list(range(N_CORES)), trace=_trace
    )
    # per-core out: [3, H, B_CORE] -> [B_CORE, 3*H]
    outs = []
    for c in range(N_CORES):
        o = res.results[c]["out"]
        outs.append(o.reshape(3 * H, B_CORE).T.reshape(B_CORE, 3 * H))
    full = np.concatenate(outs, axis=0).astype(np.float32)
    if _trace:
        return full, res
    return full
